# revision 10
# baseline (speedup 1.0000x reference)
"""LocalAutoCorr2D Trainium2 kernel.

out[b,c,i,j,dy,dx] = sum_{y,x valid} x[b,c,4i+y,4j+x] * x[b,c,4i+y+sy,4j+x+sx]
with (sy,sx) = (dy-4, dx-4), windows 8x8 at stride 4 on a 96x96 image,
zero-padded at window boundaries.

Strategy (per core, batch-sharded over 8 cores):
  - out[s] == out[-s] (autocorr symmetry) -> only 40 canonical shift classes.
  - Per class: product Q = x .* shift(x) on the Vector engine (fp16, 2x
    mode); one spatial box-sum runs on the Tensor engine as a 0/1-weight
    matmul over the partition dim; the other is folded into PSUM
    accumulation across shifted-column taps.
  - The PE streams ~1 column/cycle, so tap count is the cost. Each class
    picks the cheaper orientation: h-on-partitions (taps = 8-|sx|) or
    w-on-partitions (taps = 8-sy). The host uploads x in both layouts
    ([h,(w,c)] and [w,(h,c)]) so both orientations get contiguous fp16
    tiles; vertical/horizontal shifts become partition-shifted SBUF copies
    and 64*s column offsets (always 4B-aligned -> DVE 2x stays on).
  - Outputs leave the chip as fp16 (Act converts on the PSUM->SBUF copy);
    the host widens to fp32.
"""

import functools
import os
import sys

import numpy as np

sys.path.insert(0, "/opt/trn_rl_repo")

import concourse.bass as bass  # noqa: E402
import concourse.bacc as bacc  # noqa: E402
import concourse.mybir as mybir  # noqa: E402
from concourse import bass_utils  # noqa: E402
from concourse.tile import TileContext  # noqa: E402

B, C, H, W = 8, 64, 96, 96
KH = KW = 8
SH = SW = 4
NH = NW = 23
NCORES = 8
WC = W * C  # 6144 flat (w,c) columns
PADC = 4 * C  # column padding so sx in [-4,4] (64*sx) offsets stay in-tile

fp32 = mybir.dt.float32
fp16 = mybir.dt.float16

# classes where the flipped (w-on-partition) orientation has fewer taps
FLIP = {(sy, sx) for sy in range(1, 5) for sx in range(-2, 3) if sy > abs(sx)}
SXK = [0, 1, 2]  # bmat block order; sx<0 maps to +|sx| by operand swap


def _canonical_cells():
    """Map canonical shift (sy>=0, sx) -> list of output cells (dy,dx)."""
    cells = {}
    for dy in range(8):
        for dx in range(8):
            sy, sx = dy - 4, dx - 4
            key = (sy, sx) if (sy > 0 or (sy == 0 and sx >= 0)) else (-sy, -sx)
            cells.setdefault(key, []).append((dy, dx))
    assert len(cells) == 40
    return cells


def _amat_np():
    """Vertical box-sum matrices, stacked: A[h, sy*23+i] = 1 if 0<=h-4i<8-sy."""
    a = np.zeros((H, 5 * NH), np.float16)
    for sy in range(5):
        for i in range(NH):
            a[4 * i : 4 * i + 8 - sy, sy * NH + i] = 1.0
    return a


def _bmat_np():
    """Horizontal box-sum matrices for flipped classes:
    B[w, k*23+j] = 1 if (w-4j) in [0, 8-k). Classes with sx<0 re-index the
    product field by w' = w+sx (swapping which operand carries the partition
    shift), which lands on the same +|sx| window."""
    b = np.zeros((W, 3 * NW), np.float16)
    for k in SXK:
        for j in range(NW):
            b[4 * j : 4 * j + 8 - k, k * NW + j] = 1.0
    return b


J_CHUNKS = [(0, 8), (8, 16), (16, 23)]  # N = 512/512/448 <= 512 per matmul


def build_nc():
    nc = bacc.Bacc()
    x_dram = nc.dram_tensor("x", [H, WC], fp16, kind="ExternalInput")
    xt_dram = nc.dram_tensor("xt", [W, H * C], fp16, kind="ExternalInput")
    amat_dram = nc.dram_tensor("amat", [H, 5 * NH], fp16, kind="ExternalInput")
    bmat_dram = nc.dram_tensor("bmat", [W, 3 * NW], fp16, kind="ExternalInput")
    out_dram = nc.dram_tensor("out", [8, 8, NH, NW * C], fp16,
                              kind="ExternalOutput")

    cells = _canonical_cells()
    # order: by sy so early shifts only need T0, shifted copies land meanwhile
    order = sorted(cells.keys(), key=lambda s: (s[0], abs(s[1])))

    with TileContext(nc) as tc:
        with (
            tc.tile_pool(name="const", bufs=1) as cpool,
            tc.tile_pool(name="tcop", bufs=1) as tpool,
            tc.tile_pool(name="q", bufs=3) as qpool,
            tc.tile_pool(name="o", bufs=2) as opool,
            tc.tile_pool(name="ps", bufs=2, space="PSUM") as ppool,
        ):
            amat_t = cpool.tile([H, 5 * NH], fp16)
            bmat_t = cpool.tile([W, 3 * NW], fp16)

            # normal tiles: T[sy][r, PADC+k] = x[r+sy, k], zero-padded columns
            tt = {}
            for sy in range(5):
                t = tpool.tile([H, PADC + WC + PADC], fp16, name=f"T{sy}")
                tt[sy] = t
                nc.vector.memset(t[:, 0:PADC], 0.0)
                nc.vector.memset(t[:, PADC + WC : PADC + WC + PADC], 0.0)
            # flipped tiles: U[s][w, k] = xt[w+s, k]; no pads needed
            uu = {}
            for s in [0, 1, 2]:
                uu[s] = tpool.tile([W, H * C], fp16, name=f"U{s}")

            t0, u0 = tt[0], uu[0]
            for k in range(2):
                nc.gpsimd.dma_start(
                    t0[:, PADC + k * 3072 : PADC + (k + 1) * 3072],
                    x_dram[:, k * 3072 : (k + 1) * 3072],
                )
                nc.gpsimd.dma_start(
                    u0[:, k * 3072 : (k + 1) * 3072],
                    xt_dram[:, k * 3072 : (k + 1) * 3072],
                )
            nc.sync.dma_start(amat_t, amat_dram[:, :])
            nc.sync.dma_start(bmat_t, bmat_dram[:, :])
            # shifted copies via SBUF->SBUF DMA, spread across dispatch
            # queues so the transfers run in parallel (one queue serializes
            # them into a ~30us stall blocking the first shifted classes)
            for sy in range(1, 5):
                eng = nc.scalar if sy % 2 else nc.sync
                eng.dma_start(
                    tt[sy][0 : H - sy, PADC : PADC + WC],
                    t0[sy:H, PADC : PADC + WC],
                )
            nc.scalar.dma_start(uu[1][0 : W - 1, :], u0[1:W, :])
            nc.sync.dma_start(uu[2][0 : W - 2, :], u0[2:W, :])

            for (sy, sx) in order:
                o_t = opool.tile([NH, NW * C], fp16, tag="o")
                if (sy, sx) in FLIP:
                    # w on partitions: matmul contracts the horizontal
                    # window; taps run over the 8-sy vertical offsets.
                    # sx<0 re-indexes by w' = w+sx: the positively-shifted
                    # tile becomes in0 and the window maps to block +|sx|.
                    sxa = abs(sx)
                    whi = W - sxa
                    cw = H * C - 64 * sy
                    q = qpool.tile([W, H * C], fp16, tag="q")
                    if sx >= 0:
                        nc.vector.tensor_mul(
                            q[0:whi, 0:cw],
                            u0[0:whi, 0:cw],
                            uu[sxa][0:whi, 64 * sy : 64 * sy + cw],
                        )
                    else:
                        nc.vector.tensor_mul(
                            q[0:whi, 0:cw],
                            uu[sxa][0:whi, 0:cw],
                            u0[0:whi, 64 * sy : 64 * sy + cw],
                        )
                    qv = q.rearrange("w (h c) -> w h c", c=C)
                    b_k = bmat_t[0:whi, sxa * NW : (sxa + 1) * NW]
                    for ci, (i0, i1) in enumerate(J_CHUNKS):
                        pt = ppool.tile([NW, (i1 - i0) * C], fp32,
                                        tag=f"ps{ci}")
                        for yi in range(8 - sy):
                            rhs = qv[0:whi, yi + 4 * i0 : yi + 4 * i1 - 3 : 4, :]
                            nc.tensor.matmul(
                                pt, b_k, rhs,
                                start=(yi == 0), stop=(yi == 7 - sy),
                            )
                        nc.scalar.copy(o_t[:, i0 * C : i1 * C], pt)
                else:
                    hv = H - sy
                    q = qpool.tile([H, WC], fp16, tag="q")
                    off = PADC + 64 * sx
                    nc.vector.tensor_mul(
                        q[0:hv, :],
                        t0[0:hv, PADC : PADC + WC],
                        tt[sy][0:hv, off : off + WC],
                    )
                    qv = q.rearrange("h (w c) -> h w c", c=C)
                    a_k = amat_t[0:hv, sy * NH : (sy + 1) * NH]
                    xlist = list(range(max(0, -sx), 8 - max(0, sx)))
                    for ci, (j0, j1) in enumerate(J_CHUNKS):
                        pt = ppool.tile([NH, (j1 - j0) * C], fp32,
                                        tag=f"ps{ci}")
                        for xi, xx in enumerate(xlist):
                            rhs = qv[0:hv, xx + 4 * j0 : xx + 4 * j1 - 3 : 4, :]
                            nc.tensor.matmul(
                                pt, a_k, rhs,
                                start=(xi == 0), stop=(xi == len(xlist) - 1),
                            )
                        nc.scalar.copy(o_t[:, j0 * C : j1 * C], pt)
                for (dy, dx) in cells[(sy, sx)]:
                    nc.gpsimd.dma_start(out_dram[dy, dx], o_t)

    if not nc.is_finalized():
        nc.finalize()
    return nc


@functools.lru_cache(maxsize=1)
def _get_nc():
    return build_nc()


def _in_maps(x):
    """Per-core input dicts: x[b] in both [h,(w,c)] and [w,(h,c)] fp16."""
    amat, bmat = _amat_np(), _bmat_np()
    maps = []
    for b in range(NCORES):
        xb = x[b].astype(np.float16)  # [c, h, w]
        maps.append({
            "x": np.ascontiguousarray(xb.transpose(1, 2, 0).reshape(H, WC)),
            "xt": np.ascontiguousarray(xb.transpose(2, 1, 0).reshape(W, H * C)),
            "amat": amat,
            "bmat": bmat,
        })
    return maps


@functools.lru_cache(maxsize=1)
def _flip_cells():
    cells = _canonical_cells()
    return tuple(
        (dy, dx) for key in FLIP for (dy, dx) in cells[key]
    )


def kernel(**inputs) -> np.ndarray:
    x = np.asarray(inputs["x"], dtype=np.float32)
    assert x.shape == (B, C, H, W)
    nc = _get_nc()
    res = bass_utils.run_bass_kernel_spmd(
        nc, _in_maps(x), core_ids=list(range(NCORES)),
        trace=bool(int(os.environ.get("KERNEL_TRACE", "0"))),
    )
    outs = np.stack([r["out"] for r in res.results]).astype(np.float32)
    outs = outs.reshape(B, 8, 8, NH, NW, C)
    # flipped cells landed as [j, i, c]; swap back to [i, j, c]
    for (dy, dx) in _flip_cells():
        outs[:, dy, dx] = outs[:, dy, dx].transpose(0, 2, 1, 3)
    # -> [B, c, i, j, dy, dx]
    full = outs.transpose(0, 5, 3, 4, 1, 2)
    return np.ascontiguousarray(full).astype(np.float32)


if __name__ == "__main__":
    rng = np.random.default_rng(0)
    x = rng.standard_normal((B, C, H, W), dtype=np.float32)
    y = kernel(x=x)
    print("out", y.shape, y.dtype, float(np.abs(y).max()))


# revision 11
# speedup vs baseline: 1.2970x; 1.2970x over previous
"""LocalAutoCorr2D Trainium2 kernel.

out[b,c,i,j,dy,dx] = sum_{y,x valid} x[b,c,4i+y,4j+x] * x[b,c,4i+y+sy,4j+x+sx]
with (sy,sx) = (dy-4, dx-4), windows 8x8 at stride 4 on a 96x96 image,
zero-padded at window boundaries.

Strategy (per core, batch-sharded over 8 cores):
  - out[s] == out[-s] (autocorr symmetry) -> only 40 canonical shift classes.
  - Per class: product Q = x .* shift(x) on the Vector engine (fp16, 2x
    mode); one spatial box-sum runs on the Tensor engine as a 0/1-weight
    matmul over the partition dim; the other is folded into PSUM
    accumulation across shifted-column taps.
  - The PE streams ~1 column/cycle, so tap count is the cost. Each class
    picks the cheaper orientation: h-on-partitions (taps = 8-|sx|) or
    w-on-partitions (taps = 8-sy). The host uploads x in both layouts
    ([h,(w,c)] and [w,(h,c)]) so both orientations get contiguous fp16
    tiles; vertical/horizontal shifts become partition-shifted SBUF copies
    and 64*s column offsets (always 4B-aligned -> DVE 2x stays on).
  - Outputs leave the chip as fp16 (Act converts on the PSUM->SBUF copy);
    the host widens to fp32.
"""

import functools
import os
import sys

import numpy as np

sys.path.insert(0, "/opt/trn_rl_repo")

import concourse.bass as bass  # noqa: E402
import concourse.bacc as bacc  # noqa: E402
import concourse.mybir as mybir  # noqa: E402
from concourse import bass_utils  # noqa: E402
from concourse.tile import TileContext  # noqa: E402

B, C, H, W = 8, 64, 96, 96
KH = KW = 8
SH = SW = 4
NH = NW = 23
NCORES = 8
WC = W * C  # 6144 flat (w,c) columns
PADC = 4 * C  # column padding so sx in [-4,4] (64*sx) offsets stay in-tile

fp32 = mybir.dt.float32
fp16 = mybir.dt.float16

# classes where the flipped (w-on-partition) orientation has fewer taps
FLIP = {(sy, sx) for sy in range(1, 5) for sx in range(-2, 3) if sy > abs(sx)}
SXK = [0, 1, 2]  # bmat block order; sx<0 maps to +|sx| by operand swap


def _canonical_cells():
    """Map canonical shift (sy>=0, sx) -> list of output cells (dy,dx)."""
    cells = {}
    for dy in range(8):
        for dx in range(8):
            sy, sx = dy - 4, dx - 4
            key = (sy, sx) if (sy > 0 or (sy == 0 and sx >= 0)) else (-sy, -sx)
            cells.setdefault(key, []).append((dy, dx))
    assert len(cells) == 40
    return cells


def _amat_np():
    """Vertical box-sum matrices, stacked: A[h, sy*23+i] = 1 if 0<=h-4i<8-sy."""
    a = np.zeros((H, 5 * NH), np.float16)
    for sy in range(5):
        for i in range(NH):
            a[4 * i : 4 * i + 8 - sy, sy * NH + i] = 1.0
    return a


def _bmat_np():
    """Horizontal box-sum matrices for flipped classes:
    B[w, k*23+j] = 1 if (w-4j) in [0, 8-k). Classes with sx<0 re-index the
    product field by w' = w+sx (swapping which operand carries the partition
    shift), which lands on the same +|sx| window."""
    b = np.zeros((W, 3 * NW), np.float16)
    for k in SXK:
        for j in range(NW):
            b[4 * j : 4 * j + 8 - k, k * NW + j] = 1.0
    return b


J_CHUNKS = [(0, 8), (8, 16), (16, 23)]  # N = 512/512/448 <= 512 per matmul


def build_nc():
    nc = bacc.Bacc()
    x_dram = nc.dram_tensor("x", [H, WC], fp16, kind="ExternalInput")
    xt_dram = nc.dram_tensor("xt", [W, H * C], fp16, kind="ExternalInput")
    amat_dram = nc.dram_tensor("amat", [H, 5 * NH], fp16, kind="ExternalInput")
    bmat_dram = nc.dram_tensor("bmat", [W, 3 * NW], fp16, kind="ExternalInput")
    out_dram = nc.dram_tensor("out", [8, 8, NH, NW * C], fp16,
                              kind="ExternalOutput")

    cells = _canonical_cells()
    # order: by sy so early shifts only need T0, shifted copies land meanwhile
    order = sorted(cells.keys(), key=lambda s: (s[0], abs(s[1])))

    with TileContext(nc) as tc:
        with (
            tc.tile_pool(name="const", bufs=1) as cpool,
            tc.tile_pool(name="tcop", bufs=1) as tpool,
            tc.tile_pool(name="q", bufs=3) as qpool,
            tc.tile_pool(name="o", bufs=2) as opool,
            tc.tile_pool(name="ps", bufs=2, space="PSUM") as ppool,
        ):
            amat_t = cpool.tile([H, 5 * NH], fp16)
            bmat_t = cpool.tile([W, 3 * NW], fp16)

            # normal tiles: T[sy][r, PADC+k] = x[r+sy, k], zero-padded columns
            tt = {}
            for sy in range(5):
                t = tpool.tile([H, PADC + WC + PADC], fp16, name=f"T{sy}")
                tt[sy] = t
                nc.vector.memset(t[:, 0:PADC], 0.0)
                nc.vector.memset(t[:, PADC + WC : PADC + WC + PADC], 0.0)
            # flipped tiles: U[s][w, k] = xt[w+s, k]; no pads needed
            uu = {}
            for s in [0, 1, 2]:
                uu[s] = tpool.tile([W, H * C], fp16, name=f"U{s}")

            t0, u0 = tt[0], uu[0]
            for k in range(2):
                nc.gpsimd.dma_start(
                    t0[:, PADC + k * 3072 : PADC + (k + 1) * 3072],
                    x_dram[:, k * 3072 : (k + 1) * 3072],
                )
                nc.gpsimd.dma_start(
                    u0[:, k * 3072 : (k + 1) * 3072],
                    xt_dram[:, k * 3072 : (k + 1) * 3072],
                )
            nc.gpsimd.dma_start(amat_t, amat_dram[:, :])
            nc.gpsimd.dma_start(bmat_t, bmat_dram[:, :])
            # shifted copies via SBUF->SBUF DMA
            for sy in range(1, 5):
                nc.gpsimd.dma_start(
                    tt[sy][0 : H - sy, PADC : PADC + WC],
                    t0[sy:H, PADC : PADC + WC],
                )
            for s in [1, 2]:
                nc.gpsimd.dma_start(uu[s][0 : W - s, :], u0[s:W, :])

            for (sy, sx) in order:
                o_t = opool.tile([NH, NW * C], fp16, tag="o")
                if (sy, sx) in FLIP:
                    # w on partitions: matmul contracts the horizontal
                    # window; taps run over the 8-sy vertical offsets.
                    # sx<0 re-indexes by w' = w+sx: the positively-shifted
                    # tile becomes in0 and the window maps to block +|sx|.
                    sxa = abs(sx)
                    whi = W - sxa
                    cw = H * C - 64 * sy
                    q = qpool.tile([W, H * C], fp16, tag="q")
                    if sx >= 0:
                        nc.vector.tensor_mul(
                            q[0:whi, 0:cw],
                            u0[0:whi, 0:cw],
                            uu[sxa][0:whi, 64 * sy : 64 * sy + cw],
                        )
                    else:
                        nc.vector.tensor_mul(
                            q[0:whi, 0:cw],
                            uu[sxa][0:whi, 0:cw],
                            u0[0:whi, 64 * sy : 64 * sy + cw],
                        )
                    qv = q.rearrange("w (h c) -> w h c", c=C)
                    b_k = bmat_t[0:whi, sxa * NW : (sxa + 1) * NW]
                    for ci, (i0, i1) in enumerate(J_CHUNKS):
                        pt = ppool.tile([NW, (i1 - i0) * C], fp32,
                                        tag=f"ps{ci}")
                        for yi in range(8 - sy):
                            rhs = qv[0:whi, yi + 4 * i0 : yi + 4 * i1 - 3 : 4, :]
                            nc.tensor.matmul(
                                pt, b_k, rhs,
                                start=(yi == 0), stop=(yi == 7 - sy),
                            )
                        nc.scalar.copy(o_t[:, i0 * C : i1 * C], pt)
                else:
                    hv = H - sy
                    q = qpool.tile([H, WC], fp16, tag="q")
                    off = PADC + 64 * sx
                    nc.vector.tensor_mul(
                        q[0:hv, :],
                        t0[0:hv, PADC : PADC + WC],
                        tt[sy][0:hv, off : off + WC],
                    )
                    qv = q.rearrange("h (w c) -> h w c", c=C)
                    a_k = amat_t[0:hv, sy * NH : (sy + 1) * NH]
                    xlist = list(range(max(0, -sx), 8 - max(0, sx)))
                    for ci, (j0, j1) in enumerate(J_CHUNKS):
                        pt = ppool.tile([NH, (j1 - j0) * C], fp32,
                                        tag=f"ps{ci}")
                        for xi, xx in enumerate(xlist):
                            rhs = qv[0:hv, xx + 4 * j0 : xx + 4 * j1 - 3 : 4, :]
                            nc.tensor.matmul(
                                pt, a_k, rhs,
                                start=(xi == 0), stop=(xi == len(xlist) - 1),
                            )
                        nc.scalar.copy(o_t[:, j0 * C : j1 * C], pt)
                for (dy, dx) in cells[(sy, sx)]:
                    nc.gpsimd.dma_start(out_dram[dy, dx], o_t)

    if not nc.is_finalized():
        nc.finalize()
    return nc


@functools.lru_cache(maxsize=1)
def _get_nc():
    return build_nc()


def _in_maps(x):
    """Per-core input dicts: x[b] in both [h,(w,c)] and [w,(h,c)] fp16."""
    amat, bmat = _amat_np(), _bmat_np()
    maps = []
    for b in range(NCORES):
        xb = x[b].astype(np.float16)  # [c, h, w]
        maps.append({
            "x": np.ascontiguousarray(xb.transpose(1, 2, 0).reshape(H, WC)),
            "xt": np.ascontiguousarray(xb.transpose(2, 1, 0).reshape(W, H * C)),
            "amat": amat,
            "bmat": bmat,
        })
    return maps


@functools.lru_cache(maxsize=1)
def _flip_cells():
    cells = _canonical_cells()
    return tuple(
        (dy, dx) for key in FLIP for (dy, dx) in cells[key]
    )


def kernel(**inputs) -> np.ndarray:
    x = np.asarray(inputs["x"], dtype=np.float32)
    assert x.shape == (B, C, H, W)
    nc = _get_nc()
    res = bass_utils.run_bass_kernel_spmd(
        nc, _in_maps(x), core_ids=list(range(NCORES)),
        trace=bool(int(os.environ.get("KERNEL_TRACE", "0"))),
    )
    outs = np.stack([r["out"] for r in res.results]).astype(np.float32)
    outs = outs.reshape(B, 8, 8, NH, NW, C)
    # flipped cells landed as [j, i, c]; swap back to [i, j, c]
    for (dy, dx) in _flip_cells():
        outs[:, dy, dx] = outs[:, dy, dx].transpose(0, 2, 1, 3)
    # -> [B, c, i, j, dy, dx]
    full = outs.transpose(0, 5, 3, 4, 1, 2)
    return np.ascontiguousarray(full).astype(np.float32)


if __name__ == "__main__":
    rng = np.random.default_rng(0)
    x = rng.standard_normal((B, C, H, W), dtype=np.float32)
    y = kernel(x=x)
    print("out", y.shape, y.dtype, float(np.abs(y).max()))
